# revision 1
# baseline (speedup 1.0000x reference)
"""Sparse Adagrad (Habana-style) on 8 Trainium2 NeuronCores.

Strategy: row-shard the embedding tables (weights/moments) across the 8
cores by index range (62500 rows each, padded to 63488 = 128*496). The
host routes each valid gradient row to its owning core. On device, each
core sweeps its table shard once with large contiguous DMAs; the sparse
scatter-add (with duplicate indices) is done with one-hot matmuls on the
TensorEngine accumulating into PSUM, so duplicates sum natively.

Table layout per core: row r -> SBUF partition p = r // 496, free offset
j = r % 496 (so a [63488, 64] f32 shard is exactly a [128, 496*64] SBUF
sweep with contiguous per-partition DMA).

Per block j (the 128 rows {p*496 + j}), the host packs the gradient rows
whose local index maps to block j into up to CPB chunks of 128 "slots"
(slot -> partition). A one-hot matrix A[slot, p] = (strip(slot) == p)
is built on device via is_equal against an iota, and
    psum_m[p, :] += A.T @ g2_chunk      (moment increments, Sum g^2)
    psum_g[p, :] += A.T @ g_chunk       (gradient sums, Sum g)
Then the update (denominator uses the fully accumulated moment, and it
is constant across duplicates so it factors out of the sum):
    m' = m + psum_m
    w' = w - lr * psum_g / sqrt(m' + 1e-20)
"""

import sys

for _p in ("/opt/trn_rl_repo", "/root/.axon_site/_ro/trn_rl_repo"):
    if _p not in sys.path:
        sys.path.insert(0, _p)

import numpy as np

P = 128          # SBUF partitions / matmul contraction
D = 64           # embedding dim
NCORES = 8
VC = 62500       # table rows per core
R = 496          # rows per strip (= blocks per core); 128*496 = 63488 >= VC
PADV = P * R     # padded rows per core
JSUB = 16        # blocks per sweep iteration (PSUM limited)
NIT = R // JSUB  # 31 sweep iterations

_program_cache = {}


def _build_program(cpb, cap, jsub=JSUB, sbufs=3, pbufs=2, store_engine="scalar",
                   g_dtype="fp16", g_load_engine="sync", reps=1,
                   loop_reps=False):
    from concourse import bacc, mybir
    import concourse.tile as tile

    nit = R // jsub
    assert nit * jsub == R
    f32 = mybir.dt.float32
    nc = bacc.Bacc("TRN2", target_bir_lowering=False, debug=False,
                   num_devices=NCORES)

    w_in = nc.dram_tensor("w_in", [P, R * D], f32, kind="ExternalInput")
    m_in = nc.dram_tensor("m_in", [P, R * D], f32, kind="ExternalInput")
    gdt = {"f32": f32, "bf16": mybir.dt.bfloat16,
           "fp16": mybir.dt.float16}[g_dtype]
    g_in = nc.dram_tensor("g_in", [cap, R * cpb * D], gdt,
                          kind="ExternalInput")
    midx = nc.dram_tensor("midx", [cap, R * cpb], f32, kind="ExternalInput")
    lr_in = nc.dram_tensor("lr", [1, 1], f32, kind="ExternalInput")
    w_out = nc.dram_tensor("w_out", [P, R * D], f32, kind="ExternalOutput")
    m_out = nc.dram_tensor("m_out", [P, R * D], f32, kind="ExternalOutput")

    with tile.TileContext(nc) as tc:
        with tc.tile_pool(name="consts", bufs=1) as consts, \
             tc.tile_pool(name="sbuf", bufs=sbufs) as pool, \
             tc.tile_pool(name="psum", bufs=pbufs, space="PSUM") as psum:
            iota_i = consts.tile([P, P], mybir.dt.int32)
            nc.gpsimd.iota(iota_i[:], pattern=[[1, P]], base=0,
                           channel_multiplier=0)
            iota_f = consts.tile([P, P], f32)
            nc.vector.tensor_copy(iota_f[:], iota_i[:])

            eps_t = consts.tile([P, 1], f32)
            nc.gpsimd.memset(eps_t[:], 1e-20)

            # inv_s2 = 1 / stream_scale^2 (precomputed on host), used to
            # recover Sum g^2 from Sum (stream_scale*g)^2
            inv_s2 = consts.tile([P, 1], f32)
            nc.sync.dma_start(out=inv_s2[:], in_=lr_in[:].to_broadcast((P, 1)))

            midx_s = consts.tile([cap, R * cpb], f32)
            nc.sync.dma_start(out=midx_s[:], in_=midx[:])

            store = getattr(nc, store_engine)

            import contextlib

            def _rep_scope():
                if loop_reps and reps > 1:
                    return tc.For_i(0, reps, 1)
                return contextlib.nullcontext()

            with _rep_scope():
              for _rep in range(1 if loop_reps else reps):
                for it in range(nit):
                    c0, c1 = it * jsub * D, (it + 1) * jsub * D
                    s0, s1 = it * jsub * cpb * D, (it + 1) * jsub * cpb * D
                    k0 = it * jsub * cpb

                    w_s = pool.tile([P, jsub * D], f32)
                    nc.sync.dma_start(out=w_s[:], in_=w_in[:, c0:c1])
                    m_s = pool.tile([P, jsub * D], f32)
                    nc.sync.dma_start(out=m_s[:], in_=m_in[:, c0:c1])
                    g_s = pool.tile([cap, jsub * cpb * D], gdt)
                    getattr(nc, g_load_engine).dma_start(out=g_s[:],
                                                         in_=g_in[:, s0:s1])

                    g2_s = pool.tile([cap, jsub * cpb * D], gdt)
                    nc.scalar.square(g2_s[:], g_s[:])

                    a_s = pool.tile([cap, jsub * cpb, P], gdt)
                    nc.vector.tensor_tensor(
                        out=a_s[:],
                        in0=midx_s[:, k0:k0 + jsub * cpb, None].broadcast_to(
                            (cap, jsub * cpb, P)),
                        in1=iota_f[:cap, None, :].broadcast_to(
                            (cap, jsub * cpb, P)),
                        op=mybir.AluOpType.is_equal,
                    )

                    psum_m = psum.tile([P, jsub * D], f32)
                    psum_g = psum.tile([P, jsub * D], f32)
                    for jj in range(jsub):
                        for c in range(cpb):
                            k = jj * cpb + c
                            nc.tensor.matmul(
                                out=psum_m[:, jj * D:(jj + 1) * D],
                                lhsT=a_s[:, k, :],
                                rhs=g2_s[:, k * D:(k + 1) * D],
                                start=(c == 0), stop=(c == cpb - 1),
                            )
                        for c in range(cpb):
                            k = jj * cpb + c
                            nc.tensor.matmul(
                                out=psum_g[:, jj * D:(jj + 1) * D],
                                lhsT=a_s[:, k, :],
                                rhs=g_s[:, k * D:(k + 1) * D],
                                start=(c == 0), stop=(c == cpb - 1),
                            )

                    m_n = pool.tile([P, jsub * D], f32)
                    nc.vector.affine_then_add(out=m_n[:], in0=psum_m[:],
                                              in1=m_s[:], scale=inv_s2[:],
                                              bias=0.0)
                    store.dma_start(out=m_out[:, c0:c1], in_=m_n[:])

                    s_t = pool.tile([P, jsub * D], f32)
                    nc.scalar.activation(s_t[:], m_n[:],
                                         mybir.ActivationFunctionType.Sqrt,
                                         bias=eps_t[:])
                    r_t = pool.tile([P, jsub * D], f32)
                    nc.vector.reciprocal_approx_fast(out=r_t[:], in_=s_t[:])
                    t_t = pool.tile([P, jsub * D], f32)
                    nc.vector.tensor_mul(t_t[:], r_t[:], psum_g[:])
                    w_n = pool.tile([P, jsub * D], f32)
                    nc.gpsimd.tensor_tensor(out=w_n[:], in0=w_s[:], in1=t_t[:],
                                            op=mybir.AluOpType.add)
                    store.dma_start(out=w_out[:, c0:c1], in_=w_n[:])

    nc.compile()
    return nc


def get_program(cpb, cap, **opts):
    key = (cpb, cap, tuple(sorted(opts.items())))
    if key not in _program_cache:
        _program_cache[key] = _build_program(cpb, cap, **opts)
    return _program_cache[key]


def prepare_inputs(gradients, weights, moments, indices, learning_rate,
                   valid_count, g_dtype="fp16"):
    """Host-side routing: shard tables by row range, route gradient rows to
    owning cores, pack into the block/slot layout the device sweep expects."""
    g = np.ascontiguousarray(np.asarray(gradients, dtype=np.float32))
    w = np.asarray(weights, dtype=np.float32)
    m = np.asarray(moments, dtype=np.float32)
    idx = np.asarray(indices).astype(np.int64)
    vc = int(valid_count)
    lr = np.float32(np.asarray(learning_rate).reshape(-1)[0])

    idxv = idx[:vc]
    owner = idxv // VC
    loc = idxv - owner * VC
    j = loc % R
    mstrip = loc // R

    group = owner * R + j
    counts = np.bincount(group, minlength=NCORES * R)
    order = np.argsort(group, kind="stable")
    starts = np.concatenate(([0], np.cumsum(counts)[:-1]))
    rank = np.empty(vc, dtype=np.int64)
    rank[order] = np.arange(vc, dtype=np.int64) - starts[group[order]]

    maxcnt = max(1, int(counts.max()))
    cap = min(P, -(-maxcnt // 16) * 16)  # chunk capacity, multiple of 16
    cpb = -(-maxcnt // cap)              # chunks per block

    colidx = j * cpb + rank // cap
    part = rank % cap

    if g_dtype == "bf16":
        import ml_dtypes
        np_gdt = ml_dtypes.bfloat16
    elif g_dtype == "fp16":
        np_gdt = np.float16
    else:
        np_gdt = np.float32
    sscale = -lr if lr != 0.0 else 1.0
    g_dev = np.zeros((NCORES, cap, R * cpb, D), dtype=np_gdt)
    g_dev[owner, part, colidx] = (np.float32(sscale) * g[:vc]).astype(np_gdt)
    g_dev = g_dev.reshape(NCORES, cap, R * cpb * D)

    midx_dev = np.zeros((NCORES, cap, R * cpb), dtype=np.float32)
    midx_dev[owner, part, colidx] = mstrip.astype(np.float32)

    w_dev = np.zeros((NCORES, PADV, D), dtype=np.float32)
    w_dev[:, :VC] = w.reshape(NCORES, VC, D)
    w_dev = w_dev.reshape(NCORES, P, R * D)
    m_dev = np.zeros((NCORES, PADV, D), dtype=np.float32)
    m_dev[:, :VC] = m.reshape(NCORES, VC, D)
    m_dev = m_dev.reshape(NCORES, P, R * D)

    lr_arr = np.full((1, 1), 1.0 / (sscale * sscale), dtype=np.float32)

    in_maps = [
        {
            "w_in": w_dev[c],
            "m_in": m_dev[c],
            "g_in": g_dev[c],
            "midx": midx_dev[c],
            "lr": lr_arr,
        }
        for c in range(NCORES)
    ]
    return in_maps, cpb, cap


def assemble_outputs(results):
    w_new = np.empty((NCORES * VC, D), dtype=np.float32)
    m_new = np.empty((NCORES * VC, D), dtype=np.float32)
    for c in range(NCORES):
        w_new[c * VC:(c + 1) * VC] = \
            results[c]["w_out"].reshape(PADV, D)[:VC]
        m_new[c * VC:(c + 1) * VC] = \
            results[c]["m_out"].reshape(PADV, D)[:VC]
    return w_new, m_new


def kernel(gradients, weights, moments, indices, learning_rate, valid_count):
    from concourse.bass_utils import run_bass_kernel_spmd

    lr = float(np.asarray(learning_rate).reshape(-1)[0])
    if lr == 0.0:
        # Degenerate case (never hit with this spec): weights unchanged,
        # moments still accumulate g^2. Compute on host.
        g = np.asarray(gradients, dtype=np.float32).copy()
        g[int(valid_count):] = 0.0
        idx = np.asarray(indices).astype(np.int64)
        m_new = np.asarray(moments, dtype=np.float32).copy()
        np.add.at(m_new, idx, g * g)
        return np.asarray(weights, dtype=np.float32).copy(), m_new

    in_maps, cpb, cap = prepare_inputs(gradients, weights, moments, indices,
                                       learning_rate, valid_count)
    nc = get_program(cpb, cap)
    res = run_bass_kernel_spmd(nc, in_maps, core_ids=list(range(NCORES)))
    return assemble_outputs(res.results)

